# revision 3
# baseline (speedup 1.0000x reference)
"""CPA-loss kernel for 8 TRN2 NeuronCores.

Math: for row b with target t, the reference loss collapses to
    loss[b] = -log( e[b,t] / (dot(s[t,:], e[b,:]) + eps) + eps ),
    e = exp(z - max(z))  (the s[t,t]=1 diagonal cancels the "+e[b,i]" term).
Both e[b,t] and the dot are invariant to the max-subtraction except for the
eps scale (effect ~2e-7 on the mean loss, far below fp32 tolerance), so we
use e = exp(z) directly and never compute the row max.

Strategy: sort rows by target on the host (the mean is permutation
invariant), deal them round-robin to 8 cores. Each core gets its 16384 rows
as a transposed [100, 16384] tile (class on partitions). Consecutive sorted
rows share targets, so every 128-row block touches at most a few distinct
classes; per block we ship 2m tiny vectors (s[c,:] and onehot(c) for each of
the block's m classes) and do ONE PE matmul
    out[128, 2m] = (E^T block [100,128]).T @ V[100, 2m]
which yields every row's candidate denominator dot and candidate e[b,t].
Host-built 0/1 masks then select each row's true class candidate, and a
short batched DVE/ACT epilogue computes -log(e/(D+eps)+eps) and reduces.
"""

import sys

import numpy as np

for _p in ("/opt/trn_rl_repo",):
    if _p not in sys.path:
        sys.path.append(_p)

import concourse.bass as bass
import concourse.tile as tile
from concourse import bacc, mybir
from concourse.bass_utils import run_bass_kernel_spmd

B = 131072
C = 100
NCORES = 8
RPC = B // NCORES  # 16384 rows per core
BLK = 128  # rows per block (= one matmul stationary tile)
NBLK = RPC // BLK  # 128 blocks per core
GROUP = 16  # blocks per DMA/exp chunk -> 2048 rows
NGRP = NBLK // GROUP
EPS = 1e-6

TRACE = False  # test.py flips this to get a profiled run
LAST_RESULTS = None  # stash of the last BassKernelResults (for test.py)

_nc_cache = {}


def _build_nc(m: int, stride: int):
    """Trace the SPMD program. m = vector-pairs per block, stride = padded
    pair stride in the PSUM result tile (power-of-two-ish, divides 512)."""
    nc = bacc.Bacc("TRN2", target_bir_lowering=False, debug=False)
    f32 = mybir.dt.float32

    lt_d = nc.declare_dram_parameter("lt", [C, RPC], f32, isOutput=False)
    vs_d = nc.declare_dram_parameter("vs", [C, 2 * m * NBLK], f32, isOutput=False)
    w_d = [
        nc.declare_dram_parameter(f"w{i}", [BLK, NBLK], mybir.dt.uint8, isOutput=False)
        for i in range(max(m - 1, 1))
    ]
    out_d = nc.declare_dram_parameter("out", [BLK, 1], f32, isOutput=True)

    with tile.TileContext(nc) as tc:
        with (
            tc.tile_pool(name="const", bufs=1) as cpool,
            tc.tile_pool(name="lt", bufs=2) as ltp,
            tc.tile_pool(name="et", bufs=2) as etp,
            tc.tile_pool(name="fin", bufs=1) as fin,
            tc.tile_pool(name="res", bufs=1, space="PSUM") as resp,
        ):
            vs_sb = cpool.tile([C, 2 * m * NBLK], f32)
            nc.sync.dma_start(vs_sb[:], vs_d[:])
            w_sb = []
            for i in range(max(m - 1, 1)):
                w = cpool.tile([BLK, NBLK], mybir.dt.uint8, tag=f"w{i}")
                nc.sync.dma_start(w[:], w_d[i][:])
                w_sb.append(w)

            res = resp.tile([BLK, NBLK, stride], f32)

            for g in range(NGRP):
                ltg = ltp.tile([C, GROUP * BLK], f32)
                nc.sync.dma_start(
                    ltg[:], lt_d[:, g * GROUP * BLK : (g + 1) * GROUP * BLK]
                )
                etg = etp.tile([C, GROUP * BLK], f32)
                nc.scalar.activation(
                    etg[:], ltg[:], mybir.ActivationFunctionType.Exp
                )
                for k in range(GROUP):
                    kk = g * GROUP + k
                    nc.tensor.matmul(
                        res[:, kk, 0 : 2 * m],
                        etg[:, k * BLK : (k + 1) * BLK],
                        vs_sb[:, 2 * m * kk : 2 * m * (kk + 1)],
                        start=True,
                        stop=True,
                    )

            # ---- epilogue: select each row's candidate, -log(e/(D+eps)+eps)
            dsel = fin.tile([BLK, NBLK], f32)
            esel = fin.tile([BLK, NBLK], f32)
            if m == 1:
                nc.vector.tensor_copy(dsel[:], res[:, :, 0])
                nc.vector.tensor_copy(esel[:], res[:, :, 1])
            else:
                # base = last pair; overwrite with pair i where mask_i says so
                nc.vector.tensor_copy(dsel[:], res[:, :, 2 * (m - 1)])
                nc.vector.tensor_copy(esel[:], res[:, :, 2 * (m - 1) + 1])
                for i in range(m - 2, -1, -1):
                    nc.vector.copy_predicated(dsel[:], w_sb[i][:], res[:, :, 2 * i])
                    nc.vector.copy_predicated(
                        esel[:], w_sb[i][:], res[:, :, 2 * i + 1]
                    )

            dp = fin.tile([BLK, NBLK], f32)
            nc.vector.tensor_scalar_add(dp[:], dsel[:], EPS)
            rec = fin.tile([BLK, NBLK], f32)
            nc.vector.reciprocal(rec[:], dp[:])
            r = fin.tile([BLK, NBLK], f32)
            nc.vector.tensor_tensor(
                r[:], esel[:], rec[:], op=mybir.AluOpType.mult
            )
            rp = fin.tile([BLK, NBLK], f32)
            nc.vector.tensor_scalar_add(rp[:], r[:], EPS)
            lnr = fin.tile([BLK, NBLK], f32)
            lsum = fin.tile([BLK, 1], f32)
            nc.scalar.activation(
                lnr[:],
                rp[:],
                mybir.ActivationFunctionType.Ln,
                accum_out=lsum[:],
            )
            nc.sync.dma_start(out_d[:], lsum[:])

    nc.compile()
    return nc


def _pick_stride(m: int) -> int:
    # pair-group stride in f32 elems; must divide the 512-f32 PSUM bank
    for st in (2, 4, 8, 16):
        if st >= 2 * m and 512 % st == 0:
            return st
    raise ValueError(f"too many classes per block: m={m}")


def kernel(logits, s, targets):
    global LAST_RESULTS
    logits = np.asarray(logits, dtype=np.float32)
    s = np.asarray(s, dtype=np.float32)
    t = np.asarray(targets).astype(np.int64).ravel()
    assert logits.shape == (B, C) and s.shape == (C, C) and t.shape == (B,)

    order = np.argsort(t, kind="stable")
    eye = np.eye(C, dtype=np.float32)

    # per-core index sets (round-robin over globally sorted rows)
    idxs = [order[mm::NCORES] for mm in range(NCORES)]

    # classes per block: blocks are rows [128k, 128(k+1)) of the sorted core
    # slice; count the max distinct classes any block touches
    m = 1
    block_classes = []
    for idx in idxs:
        tb = t[idx].reshape(NBLK, BLK)
        cs = [np.unique(row) for row in tb]
        m = max(m, max(len(u) for u in cs))
        block_classes.append((tb, cs))
    stride = _pick_stride(m)

    in_maps = []
    for core in range(NCORES):
        idx = idxs[core]
        tb, cs = block_classes[core]
        lt = np.ascontiguousarray(logits[idx].T)  # [100, 16384]
        vs = np.empty((C, 2 * m * NBLK), dtype=np.float32)
        cmat = np.empty((m, NBLK), dtype=np.int64)
        for k in range(NBLK):
            u = cs[k]
            cmat[: len(u), k] = u
            cmat[len(u) :, k] = u[-1]
        for i in range(m):
            vs[:, 2 * i :: 2 * m] = s[cmat[i]].T
            vs[:, 2 * i + 1 :: 2 * m] = eye[cmat[i]].T
        im = {"lt": lt, "vs": vs}
        nw = max(m - 1, 1)
        for i in range(nw):
            wi = (tb == cmat[i][:, None]).T.astype(np.uint8)  # [BLK, NBLK]
            im[f"w{i}"] = np.ascontiguousarray(wi)
        in_maps.append(im)

    key = (m, stride)
    if key not in _nc_cache:
        _nc_cache[key] = _build_nc(m, stride)
    nc = _nc_cache[key]

    res = run_bass_kernel_spmd(
        nc, in_maps, core_ids=list(range(NCORES)), trace=TRACE
    )
    LAST_RESULTS = res
    total = sum(float(r["out"].sum(dtype=np.float64)) for r in res.results)
    return np.float32(-total / B)


# revision 5
# speedup vs baseline: 1.2478x; 1.2478x over previous
"""CPA-loss kernel for 8 TRN2 NeuronCores.

Math: for row b with target t, the reference loss collapses to
    loss[b] = -log( e[b,t] / (dot(s[t,:], e[b,:]) + eps) + eps ),
    e = exp(z - max(z))  (the s[t,t]=1 diagonal cancels the "+e[b,i]" term).
Both e[b,t] and the dot are invariant to the max-subtraction except for the
eps scale (effect ~2e-7 on the mean loss, far below fp32 tolerance), so we
use e = exp(z) directly and never compute the row max.

Strategy: sort rows by target on the host (the mean is permutation
invariant), deal them round-robin to 8 cores. Each core gets its 16384 rows
as a transposed [100, 16384] tile (class on partitions). Consecutive sorted
rows share targets, so every 128-row block touches at most a few distinct
classes; per block we ship the m candidate s[c,:] columns and do ONE PE
matmul  out[128, m] = (E^T block [100,128]).T @ V[100, m]  giving every
row's candidate denominator dot. Host-built 0/1 masks select each row's
true class candidate. The numerator e[b,t] = exp(logits[b, t_b]) comes from
a host-gathered z_t column (pure index selection), exp'd on device. A short
batched DVE/ACT epilogue computes -log(e_t/(D+eps)+eps) and reduces.
"""

import sys

import numpy as np

for _p in ("/opt/trn_rl_repo",):
    if _p not in sys.path:
        sys.path.append(_p)

import concourse.bass as bass
import concourse.tile as tile
from concourse import bacc, mybir
from concourse.bass_utils import run_bass_kernel_spmd

B = 131072
C = 100
NCORES = 8
RPC = B // NCORES  # 16384 rows per core
BLK = 128  # rows per block (= one matmul stationary tile)
NBLK = RPC // BLK  # 128 blocks per core
GROUP = 16  # blocks per DMA/exp chunk -> 2048 rows
NGRP = NBLK // GROUP
EPS = 1e-6

TRACE = False  # test.py flips this to get a profiled run
LAST_RESULTS = None  # stash of the last BassKernelResults (for test.py)

_nc_cache = {}


def _build_nc(m: int, stride: int):
    """Trace the SPMD program. m = candidate s-columns per block, stride =
    padded candidate stride in the PSUM result tile (divides 512)."""
    nc = bacc.Bacc("TRN2", target_bir_lowering=False, debug=False)
    f32 = mybir.dt.float32
    f32r = mybir.dt.float32r

    lt_d = nc.declare_dram_parameter("lt", [C, RPC], f32, isOutput=False)
    vs_d = nc.declare_dram_parameter("vs", [C, m * NBLK], f32r, isOutput=False)
    zt_d = nc.declare_dram_parameter("zt", [BLK, NBLK], f32, isOutput=False)
    w_d = [
        nc.declare_dram_parameter(f"w{i}", [BLK, NBLK], mybir.dt.uint8, isOutput=False)
        for i in range(max(m - 1, 1))
    ]
    out_d = nc.declare_dram_parameter("out", [BLK, 1], f32, isOutput=True)

    with tile.TileContext(nc) as tc:
        with (
            tc.tile_pool(name="const", bufs=1) as cpool,
            tc.tile_pool(name="lt", bufs=2) as ltp,
            tc.tile_pool(name="et", bufs=2) as etp,
            tc.tile_pool(name="fin", bufs=1) as fin,
            tc.tile_pool(name="res", bufs=1, space="PSUM") as resp,
        ):
            vs_sb = cpool.tile([C, m * NBLK], f32r)
            nc.sync.dma_start(vs_sb[:], vs_d[:])
            zt_sb = cpool.tile([BLK, NBLK], f32)
            nc.sync.dma_start(zt_sb[:], zt_d[:])
            w_sb = []
            for i in range(max(m - 1, 1)):
                w = cpool.tile([BLK, NBLK], mybir.dt.uint8, tag=f"w{i}")
                nc.sync.dma_start(w[:], w_d[i][:])
                w_sb.append(w)

            res = resp.tile([BLK, NBLK, stride], f32)

            for g in range(NGRP):
                ltg = ltp.tile([C, GROUP * BLK], f32)
                nc.sync.dma_start(
                    ltg[:], lt_d[:, g * GROUP * BLK : (g + 1) * GROUP * BLK]
                )
                etg = etp.tile([C, GROUP * BLK], f32r)
                nc.scalar.activation(
                    etg[:], ltg[:], mybir.ActivationFunctionType.Exp
                )
                for k in range(GROUP):
                    kk = g * GROUP + k
                    nc.tensor.matmul(
                        res[:, kk, 0:m],
                        etg[:, k * BLK : (k + 1) * BLK],
                        vs_sb[:, m * kk : m * (kk + 1)],
                        start=True,
                        stop=True,
                    )

            # ---- epilogue: select candidate, -log(exp(zt)/(D+eps)+eps)
            dsel = fin.tile([BLK, NBLK], f32)
            if m == 1:
                nc.vector.tensor_copy(dsel[:], res[:, :, 0])
            else:
                # base = last candidate; overwrite with i where mask_i set
                nc.vector.tensor_copy(dsel[:], res[:, :, m - 1])
                for i in range(m - 2, -1, -1):
                    nc.vector.copy_predicated(dsel[:], w_sb[i][:], res[:, :, i])

            et = fin.tile([BLK, NBLK], f32)
            nc.scalar.activation(et[:], zt_sb[:], mybir.ActivationFunctionType.Exp)
            dp = fin.tile([BLK, NBLK], f32)
            nc.vector.tensor_scalar_add(dp[:], dsel[:], EPS)
            rec = fin.tile([BLK, NBLK], f32)
            nc.vector.reciprocal(rec[:], dp[:])
            r = fin.tile([BLK, NBLK], f32)
            nc.vector.tensor_tensor(
                r[:], et[:], rec[:], op=mybir.AluOpType.mult
            )
            rp = fin.tile([BLK, NBLK], f32)
            nc.vector.tensor_scalar_add(rp[:], r[:], EPS)
            lnr = fin.tile([BLK, NBLK], f32)
            lsum = fin.tile([BLK, 1], f32)
            nc.scalar.activation(
                lnr[:],
                rp[:],
                mybir.ActivationFunctionType.Ln,
                accum_out=lsum[:],
            )
            nc.sync.dma_start(out_d[:], lsum[:])

    nc.compile()
    return nc


def _pick_stride(m: int) -> int:
    # candidate-group stride in f32 elems; must divide the 512-f32 PSUM bank
    for st in (1, 2, 4, 8, 16):
        if st >= m and 512 % st == 0:
            return st
    raise ValueError(f"too many classes per block: m={m}")


def kernel(logits, s, targets):
    global LAST_RESULTS
    logits = np.asarray(logits, dtype=np.float32)
    s = np.asarray(s, dtype=np.float32)
    t = np.asarray(targets).astype(np.int64).ravel()
    assert logits.shape == (B, C) and s.shape == (C, C) and t.shape == (B,)

    order = np.argsort(t, kind="stable")
    zt_all = logits[np.arange(B), t]  # host gather of logits[b, t_b]

    # per-core index sets (round-robin over globally sorted rows)
    idxs = [order[mm::NCORES] for mm in range(NCORES)]

    # classes per block: blocks are rows [128k, 128(k+1)) of the sorted core
    # slice; count the max distinct classes any block touches
    m = 1
    block_classes = []
    for idx in idxs:
        tb = t[idx].reshape(NBLK, BLK)
        cs = [np.unique(row) for row in tb]
        m = max(m, max(len(u) for u in cs))
        block_classes.append((tb, cs))
    stride = _pick_stride(m)

    in_maps = []
    for core in range(NCORES):
        idx = idxs[core]
        tb, cs = block_classes[core]
        lt = np.ascontiguousarray(logits[idx].T)  # [100, 16384]
        zt = np.ascontiguousarray(zt_all[idx].reshape(NBLK, BLK).T)  # [BLK,NBLK]
        vs = np.empty((C, m * NBLK), dtype=np.float32)
        cmat = np.empty((m, NBLK), dtype=np.int64)
        for k in range(NBLK):
            u = cs[k]
            cmat[: len(u), k] = u
            cmat[len(u) :, k] = u[-1]
        for i in range(m):
            vs[:, i::m] = s[cmat[i]].T
        im = {"lt": lt, "vs": vs, "zt": zt}
        nw = max(m - 1, 1)
        for i in range(nw):
            wi = (tb == cmat[i][:, None]).T.astype(np.uint8)  # [BLK, NBLK]
            im[f"w{i}"] = np.ascontiguousarray(wi)
        in_maps.append(im)

    key = (m, stride)
    if key not in _nc_cache:
        _nc_cache[key] = _build_nc(m, stride)
    nc = _nc_cache[key]

    res = run_bass_kernel_spmd(
        nc, in_maps, core_ids=list(range(NCORES)), trace=TRACE
    )
    LAST_RESULTS = res
    total = sum(float(r["out"].sum(dtype=np.float64)) for r in res.results)
    return np.float32(-total / B)


# revision 8
# speedup vs baseline: 1.3089x; 1.0490x over previous
"""CPA-loss kernel for 8 TRN2 NeuronCores.

Math: for row b with target t, the reference loss collapses to
    loss[b] = -log( e[b,t] / (dot(s[t,:], e[b,:]) + eps) + eps ),
    e = exp(z - max(z))  (the s[t,t]=1 diagonal cancels the "+e[b,i]" term).
Both e[b,t] and the dot are invariant to the max-subtraction except for the
eps scale (effect ~2e-7 on the mean loss, far below fp32 tolerance), so we
use e = exp(z) directly and never compute the row max.

Strategy: sort rows by target on the host (the mean is permutation
invariant), deal them round-robin to 8 cores. Each core gets its 16384 rows
as a transposed [100, 16384] tile (class on partitions). Consecutive sorted
rows share targets, so every 128-row block touches at most a few distinct
classes; per block we ship the m candidate s[c,:] columns and do ONE PE
matmul  out[128, m] = (E^T block [100,128]).T @ V[100, m]  giving every
row's candidate denominator dot. Host-built 0/1 masks select each row's
true class candidate. The numerator e[b,t] = exp(logits[b, t_b]) comes from
a host-gathered z_t column (pure index selection), exp'd on device. A short
batched DVE/ACT epilogue computes -log(e_t/(D+eps)+eps) and reduces.
"""

import sys

import numpy as np

for _p in ("/opt/trn_rl_repo",):
    if _p not in sys.path:
        sys.path.append(_p)

import concourse.bass as bass
import concourse.tile as tile
from concourse import bacc, mybir
from concourse.bass_utils import run_bass_kernel_spmd

B = 131072
C = 100
NCORES = 8
RPC = B // NCORES  # 16384 rows per core
BLK = 128  # rows per block (= one matmul stationary tile)
NBLK = RPC // BLK  # 128 blocks per core
GROUP = 16  # blocks per DMA/exp chunk -> 2048 rows
NGRP = NBLK // GROUP
EPS = 1e-6

TRACE = False  # test.py flips this to get a profiled run
LAST_RESULTS = None  # stash of the last BassKernelResults (for test.py)

_nc_cache = {}


def _build_nc(m: int, stride: int):
    """Trace the SPMD program. m = candidate s-columns per block, stride =
    padded candidate stride in the PSUM result tile (divides 512)."""
    nc = bacc.Bacc("TRN2", target_bir_lowering=False, debug=False)
    f32 = mybir.dt.float32
    f32r = mybir.dt.float32r

    lt_d = nc.declare_dram_parameter("lt", [C, RPC], f32, isOutput=False)
    vs_d = nc.declare_dram_parameter("vs", [C, m * NBLK], f32r, isOutput=False)
    zt_d = nc.declare_dram_parameter("zt", [BLK, NBLK], f32, isOutput=False)
    w_d = [
        nc.declare_dram_parameter(f"w{i}", [BLK, NBLK], mybir.dt.uint8, isOutput=False)
        for i in range(max(m - 1, 1))
    ]
    out_d = nc.declare_dram_parameter("out", [BLK, 4], f32, isOutput=True)

    # group sizes for DMA/exp chunks: small first groups fill the pipeline
    # fast, then steady-state; epilogue is emitted in SLICES column-slices,
    # each with its own PSUM bank so it overlaps the matmul stream.
    gsizes = [4, 4, 8] + [16] * 7
    assert sum(gsizes) == NBLK
    SLICES = 4
    SBLK = NBLK // SLICES

    with tile.TileContext(nc) as tc:
        with (
            tc.tile_pool(name="const", bufs=1) as cpool,
            tc.tile_pool(name="lt", bufs=3) as ltp,
            tc.tile_pool(name="et", bufs=3) as etp,
            tc.tile_pool(name="fin", bufs=1) as fin,
            tc.tile_pool(name="res", bufs=1, space="PSUM") as resp,
        ):
            # first logits chunk before anything else
            lt0 = ltp.tile([C, gsizes[0] * BLK], f32, tag="lt")
            nc.sync.dma_start(lt0[:], lt_d[:, 0 : gsizes[0] * BLK])
            vs_sb = cpool.tile([C, m * NBLK], f32r)
            nc.sync.dma_start(vs_sb[:], vs_d[:])
            zt_sb = cpool.tile([BLK, NBLK], f32)
            nc.sync.dma_start(zt_sb[:], zt_d[:])
            w_sb = []
            for i in range(max(m - 1, 1)):
                w = cpool.tile([BLK, NBLK], mybir.dt.uint8, tag=f"w{i}")
                nc.sync.dma_start(w[:], w_d[i][:])
                w_sb.append(w)

            res = [
                resp.tile([BLK, SBLK, stride], f32, tag=f"res{i}", name=f"res{i}")
                for i in range(SLICES)
            ]
            lsum = fin.tile([BLK, SLICES], f32)

            def epilogue(sl):
                """select candidate, -log(exp(zt)/(D+eps)+eps) for slice sl"""
                cols = slice(sl * SBLK, (sl + 1) * SBLK)
                rsl = res[sl]
                dsel = fin.tile([BLK, SBLK], f32, tag="dsel")
                if m == 1:
                    nc.vector.tensor_copy(dsel[:], rsl[:, :, 0])
                else:
                    nc.vector.tensor_copy(dsel[:], rsl[:, :, m - 1])
                    for i in range(m - 2, -1, -1):
                        nc.vector.copy_predicated(
                            dsel[:], w_sb[i][:, cols], rsl[:, :, i]
                        )
                et = fin.tile([BLK, SBLK], f32, tag="et")
                nc.scalar.activation(
                    et[:], zt_sb[:, cols], mybir.ActivationFunctionType.Exp
                )
                dp = fin.tile([BLK, SBLK], f32, tag="dp")
                nc.vector.tensor_scalar_add(dp[:], dsel[:], EPS)
                rec = fin.tile([BLK, SBLK], f32, tag="rec")
                nc.vector.reciprocal(rec[:], dp[:])
                r = fin.tile([BLK, SBLK], f32, tag="r")
                nc.vector.tensor_tensor(
                    r[:], et[:], rec[:], op=mybir.AluOpType.mult
                )
                rp = fin.tile([BLK, SBLK], f32, tag="rp")
                nc.vector.tensor_scalar_add(rp[:], r[:], EPS)
                lnr = fin.tile([BLK, SBLK], f32, tag="lnr")
                nc.scalar.activation(
                    lnr[:],
                    rp[:],
                    mybir.ActivationFunctionType.Ln,
                    accum_out=lsum[:, sl : sl + 1],
                )

            kk = 0
            done = 0
            for g, gs in enumerate(gsizes):
                base = sum(gsizes[:g]) * BLK
                if g == 0:
                    ltg = lt0
                else:
                    ltg = ltp.tile([C, gs * BLK], f32, tag="lt")
                    nc.sync.dma_start(ltg[:], lt_d[:, base : base + gs * BLK])
                etg = etp.tile([C, gs * BLK], f32r, tag="et")
                nc.scalar.activation(
                    etg[:], ltg[:], mybir.ActivationFunctionType.Exp
                )
                for k in range(gs):
                    sl, j = kk // SBLK, kk % SBLK
                    nc.tensor.matmul(
                        res[sl][:, j, 0:m],
                        etg[:, k * BLK : (k + 1) * BLK],
                        vs_sb[:, m * kk : m * (kk + 1)],
                        start=True,
                        stop=True,
                    )
                    kk += 1
                while done < SLICES and kk >= (done + 1) * SBLK:
                    epilogue(done)
                    done += 1
            while done < SLICES:
                epilogue(done)
                done += 1

            nc.sync.dma_start(out_d[:], lsum[:])

    nc.compile()
    return nc


def _pick_stride(m: int) -> int:
    # candidate-group stride in f32 elems; must divide the 512-f32 PSUM bank
    for st in (1, 2, 4, 8, 16):
        if st >= m and 512 % st == 0:
            return st
    raise ValueError(f"too many classes per block: m={m}")


def kernel(logits, s, targets):
    global LAST_RESULTS
    logits = np.asarray(logits, dtype=np.float32)
    s = np.asarray(s, dtype=np.float32)
    t = np.asarray(targets).astype(np.int64).ravel()
    assert logits.shape == (B, C) and s.shape == (C, C) and t.shape == (B,)

    order = np.argsort(t, kind="stable")
    zt_all = logits[np.arange(B), t]  # host gather of logits[b, t_b]

    # per-core index sets (round-robin over globally sorted rows)
    idxs = [order[mm::NCORES] for mm in range(NCORES)]

    # classes per block: blocks are rows [128k, 128(k+1)) of the sorted core
    # slice; count the max distinct classes any block touches
    m = 1
    block_classes = []
    for idx in idxs:
        tb = t[idx].reshape(NBLK, BLK)
        cs = [np.unique(row) for row in tb]
        m = max(m, max(len(u) for u in cs))
        block_classes.append((tb, cs))
    stride = _pick_stride(m)

    in_maps = []
    for core in range(NCORES):
        idx = idxs[core]
        tb, cs = block_classes[core]
        lt = np.ascontiguousarray(logits[idx].T)  # [100, 16384]
        zt = np.ascontiguousarray(zt_all[idx].reshape(NBLK, BLK).T)  # [BLK,NBLK]
        vs = np.empty((C, m * NBLK), dtype=np.float32)
        cmat = np.empty((m, NBLK), dtype=np.int64)
        for k in range(NBLK):
            u = cs[k]
            cmat[: len(u), k] = u
            cmat[len(u) :, k] = u[-1]
        for i in range(m):
            vs[:, i::m] = s[cmat[i]].T
        im = {"lt": lt, "vs": vs, "zt": zt}
        nw = max(m - 1, 1)
        for i in range(nw):
            wi = (tb == cmat[i][:, None]).T.astype(np.uint8)  # [BLK, NBLK]
            im[f"w{i}"] = np.ascontiguousarray(wi)
        in_maps.append(im)

    key = (m, stride)
    if key not in _nc_cache:
        _nc_cache[key] = _build_nc(m, stride)
    nc = _nc_cache[key]

    res = run_bass_kernel_spmd(
        nc, in_maps, core_ids=list(range(NCORES)), trace=TRACE
    )
    LAST_RESULTS = res
    total = sum(float(r["out"].sum(dtype=np.float64)) for r in res.results)
    return np.float32(-total / B)


# revision 12
# speedup vs baseline: 1.3925x; 1.0639x over previous
"""CPA-loss kernel for 8 TRN2 NeuronCores.

Math: for row b with target t, the reference loss collapses to
    loss[b] = -log( e[b,t] / (dot(s[t,:], e[b,:]) + eps) + eps ),
    e = exp(z - max(z))  (the s[t,t]=1 diagonal cancels the "+e[b,i]" term).
Both e[b,t] and the dot are invariant to the max-subtraction except for the
eps scale (effect ~2e-7 on the mean loss, far below fp32 tolerance), so we
use e = exp(z) directly and never compute the row max.

Strategy: sort rows by target on the host (the mean is permutation
invariant), deal them round-robin to 8 cores. Each core gets its 16384 rows
as a transposed [100, 16384] tile (class on partitions). Consecutive sorted
rows share targets, so every 128-row block touches at most a few distinct
classes; per block we ship the m candidate s[c,:] columns and do ONE PE
matmul  out[128, m] = (E^T block [100,128]).T @ V[100, m]  giving every
row's candidate denominator dot. Host-built 0/1 masks select each row's
true class candidate. The numerator e[b,t] = exp(logits[b, t_b]) comes from
a host-gathered z_t column (pure index selection), exp'd on device. A short
batched DVE/ACT epilogue computes -log(e_t/(D+eps)+eps) and reduces.
"""

import sys

import ml_dtypes
import numpy as np

for _p in ("/opt/trn_rl_repo",):
    if _p not in sys.path:
        sys.path.append(_p)

import concourse.bass as bass
import concourse.tile as tile
from concourse import bacc, mybir
from concourse.bass_utils import run_bass_kernel_spmd

B = 131072
C = 100
NCORES = 8
RPC = B // NCORES  # 16384 rows per core
BLK = 128  # rows per block (= one matmul stationary tile)
NBLK = RPC // BLK  # 128 blocks per core
GROUP = 16  # blocks per DMA/exp chunk -> 2048 rows
NGRP = NBLK // GROUP
EPS = 1e-6

TRACE = False  # test.py flips this to get a profiled run
LAST_RESULTS = None  # stash of the last BassKernelResults (for test.py)

_nc_cache = {}


def _build_nc(m: int, stride: int):
    """Trace the SPMD program. m = candidate s-columns per block, stride =
    padded candidate stride in the PSUM result tile (divides 512)."""
    nc = bacc.Bacc("TRN2", target_bir_lowering=False, debug=False)
    f32 = mybir.dt.float32
    f32r = mybir.dt.float32r

    lt_d = nc.declare_dram_parameter("lt", [C, RPC], f32, isOutput=False)
    vs_d = nc.declare_dram_parameter("vs", [C, m * NBLK], mybir.dt.bfloat16, isOutput=False)
    zt_d = nc.declare_dram_parameter("zt", [BLK, NBLK], f32, isOutput=False)
    w_d = [
        nc.declare_dram_parameter(f"w{i}", [BLK, NBLK], mybir.dt.uint8, isOutput=False)
        for i in range(max(m - 1, 1))
    ]
    out_d = nc.declare_dram_parameter("out", [BLK, 1], f32, isOutput=True)

    # group sizes for DMA/exp chunks: small first groups fill the pipeline
    # fast, then steady-state; epilogue is emitted in SLICES column-slices,
    # each with its own PSUM bank so it overlaps the matmul stream.
    gsizes = [4, 4, 8] + [16] * 7
    assert sum(gsizes) == NBLK
    SLICES = 4
    SBLK = NBLK // SLICES

    with tile.TileContext(nc) as tc:
        with (
            tc.tile_pool(name="const", bufs=1) as cpool,
            tc.tile_pool(name="lt", bufs=3) as ltp,
            tc.tile_pool(name="et", bufs=3) as etp,
            tc.tile_pool(name="fin", bufs=1) as fin,
            tc.tile_pool(name="res", bufs=1, space="PSUM") as resp,
        ):
            # first logits chunk before anything else
            lt0 = ltp.tile([C, gsizes[0] * BLK], f32, tag="lt")
            nc.sync.dma_start(lt0[:], lt_d[:, 0 : gsizes[0] * BLK])
            vs_sb = cpool.tile([C, m * NBLK], mybir.dt.bfloat16)
            nc.sync.dma_start(vs_sb[:], vs_d[:])
            zt_sb = cpool.tile([BLK, NBLK], f32)
            nc.sync.dma_start(zt_sb[:], zt_d[:])
            w_sb = []
            for i in range(max(m - 1, 1)):
                w = cpool.tile([BLK, NBLK], mybir.dt.uint8, tag=f"w{i}")
                nc.sync.dma_start(w[:], w_d[i][:])
                w_sb.append(w)

            res = [
                resp.tile([BLK, SBLK, stride], f32, tag=f"res{i}", name=f"res{i}")
                for i in range(SLICES)
            ]
            rp_full = fin.tile([BLK, NBLK], f32)

            def epilogue(sl):
                """select candidate, exp(zt)/(D+eps)+eps for slice sl.
                Ln happens once at the end (one ACT table switch)."""
                cols = slice(sl * SBLK, (sl + 1) * SBLK)
                rsl = res[sl]
                dsel = fin.tile([BLK, SBLK], f32, tag="dsel")
                if m == 1:
                    nc.vector.tensor_copy(dsel[:], rsl[:, :, 0])
                else:
                    nc.vector.tensor_copy(dsel[:], rsl[:, :, m - 1])
                    for i in range(m - 2, -1, -1):
                        nc.vector.copy_predicated(
                            dsel[:], w_sb[i][:, cols], rsl[:, :, i]
                        )
                et = fin.tile([BLK, SBLK], f32, tag="et")
                nc.scalar.activation(
                    et[:], zt_sb[:, cols], mybir.ActivationFunctionType.Exp
                )
                dp = fin.tile([BLK, SBLK], f32, tag="dp")
                nc.vector.tensor_scalar_add(dp[:], dsel[:], EPS)
                rec = fin.tile([BLK, SBLK], f32, tag="rec")
                nc.vector.reciprocal(rec[:], dp[:])
                r = fin.tile([BLK, SBLK], f32, tag="r")
                nc.vector.tensor_tensor(
                    r[:], et[:], rec[:], op=mybir.AluOpType.mult
                )
                nc.vector.tensor_scalar_add(rp_full[:, cols], r[:], EPS)

            kk = 0
            done = 0
            for g, gs in enumerate(gsizes):
                base = sum(gsizes[:g]) * BLK
                if g == 0:
                    ltg = lt0
                else:
                    ltg = ltp.tile([C, gs * BLK], f32, tag="lt")
                    nc.sync.dma_start(ltg[:], lt_d[:, base : base + gs * BLK])
                etg = etp.tile([C, gs * BLK], mybir.dt.bfloat16, tag="et")
                nc.scalar.activation(
                    etg[:], ltg[:], mybir.ActivationFunctionType.Exp
                )
                for k in range(gs):
                    sl, j = kk // SBLK, kk % SBLK
                    nc.tensor.matmul(
                        res[sl][:, j, 0:m],
                        etg[:, k * BLK : (k + 1) * BLK],
                        vs_sb[:, m * kk : m * (kk + 1)],
                        start=True,
                        stop=True,
                    )
                    kk += 1
                while done < SLICES and kk >= (done + 1) * SBLK:
                    epilogue(done)
                    done += 1
            while done < SLICES:
                epilogue(done)
                done += 1

            lnr = fin.tile([BLK, NBLK], f32)
            lsum = fin.tile([BLK, 1], f32)
            nc.scalar.activation(
                lnr[:],
                rp_full[:],
                mybir.ActivationFunctionType.Ln,
                accum_out=lsum[:],
            )
            nc.sync.dma_start(out_d[:], lsum[:])

    nc.compile()
    return nc


def _pick_stride(m: int) -> int:
    # candidate-group stride in f32 elems; must divide the 512-f32 PSUM bank
    for st in (1, 2, 4, 8, 16):
        if st >= m and 512 % st == 0:
            return st
    raise ValueError(f"too many classes per block: m={m}")


def kernel(logits, s, targets):
    global LAST_RESULTS
    logits = np.asarray(logits, dtype=np.float32)
    s = np.asarray(s, dtype=np.float32)
    t = np.asarray(targets).astype(np.int64).ravel()
    assert logits.shape == (B, C) and s.shape == (C, C) and t.shape == (B,)

    order = np.argsort(t, kind="stable")
    zt_all = logits[np.arange(B), t]  # host gather of logits[b, t_b]

    # per-core index sets (round-robin over globally sorted rows)
    idxs = [order[mm::NCORES] for mm in range(NCORES)]

    # classes per block: blocks are rows [128k, 128(k+1)) of the sorted core
    # slice; count the max distinct classes any block touches
    m = 1
    block_classes = []
    for idx in idxs:
        tb = t[idx].reshape(NBLK, BLK)
        cs = [np.unique(row) for row in tb]
        m = max(m, max(len(u) for u in cs))
        block_classes.append((tb, cs))
    stride = _pick_stride(m)

    in_maps = []
    for core in range(NCORES):
        idx = idxs[core]
        tb, cs = block_classes[core]
        lt = np.ascontiguousarray(logits[idx].T)  # [100, 16384]
        zt = np.ascontiguousarray(zt_all[idx].reshape(NBLK, BLK).T)  # [BLK,NBLK]
        vs = np.empty((C, m * NBLK), dtype=ml_dtypes.bfloat16)
        cmat = np.empty((m, NBLK), dtype=np.int64)
        for k in range(NBLK):
            u = cs[k]
            cmat[: len(u), k] = u
            cmat[len(u) :, k] = u[-1]
        for i in range(m):
            vs[:, i::m] = s[cmat[i]].T.astype(ml_dtypes.bfloat16)
        im = {"lt": lt, "vs": vs, "zt": zt}
        nw = max(m - 1, 1)
        for i in range(nw):
            wi = (tb == cmat[i][:, None]).T.astype(np.uint8)  # [BLK, NBLK]
            im[f"w{i}"] = np.ascontiguousarray(wi)
        in_maps.append(im)

    key = (m, stride)
    if key not in _nc_cache:
        _nc_cache[key] = _build_nc(m, stride)
    nc = _nc_cache[key]

    res = run_bass_kernel_spmd(
        nc, in_maps, core_ids=list(range(NCORES)), trace=TRACE
    )
    LAST_RESULTS = res
    total = sum(float(r["out"].sum(dtype=np.float64)) for r in res.results)
    return np.float32(-total / B)
